# revision 1
# baseline (speedup 1.0000x reference)
"""EncodingGraphTransformer on 8 TRN2 NeuronCores (SPMD, full inputs in/out).

Sharding: nodes+edges dst-sharded 8 ways. Per layer: each device computes
bf16 [k|v] rows for its own (permuted) nodes, all-gathers the table, fetches
per-edge k/v with transpose-mode dma_gather (256B rows; signed-int16 window
trick -> 2 table chunks), computes attention feature-major with q read via
stride-0 broadcast over each destination's slot segment, and produces the
per-node sums with uniform-length segmented reductions over a degree-classed
slot layout.  The scalar edge feature enters algebraically:
    alpha = s*q.(k + ea*W_e) = s*q.k + ea*(s*q.W_e)   (qw rows in q table)
    msg   = ex*(v + ea*W_e)  -> out = sum(ex*v) + W_e*sum(ex*ea)
so per-edge edge-features are never materialized.  Softmax max-subtraction
is dropped (mathematically identity; alpha is O(1) here).
"""
import os
import numpy as np
import ml_dtypes

import concourse.bass as bass
import concourse.bacc as bacc
import concourse.mybir as mybir
import concourse.tile as tile
from concourse.bass_utils import run_bass_kernel_spmd
from concourse.masks import make_identity

N = 100000
E = 800000
IN_DIM = 32
HID = 64
HEADS = 4
HD = HID // HEADS
NL = 3
G = 128
NDEV = 8
NOWN = N // NDEV
SCALE = float(1.0 / np.sqrt(np.float32(HD)))
STRIP = 3072
TAIL = 128
SP = STRIP + TAIL
TW = 512
F32 = mybir.dt.float32
BF16 = mybir.dt.bfloat16
I16 = mybir.dt.int16
AX = mybir.AxisListType.X
OP = mybir.AluOpType
AF = mybir.ActivationFunctionType

CLASSES = list(range(1, 17)) + [20, 24, 28, 32, 48, 64, 96, 128]


def _class_of(m):
    for c in CLASSES:
        if m <= c:
            return c
    raise ValueError(f"degree {m} too large")


def _wrap16(idx):
    """[n] -> [128, n/16] int16: item i at [i%16, i//16], replicated x8."""
    n = len(idx)
    w = np.asarray(idx).reshape(n // 16, 16).T.astype(np.int16)
    return np.tile(w, (8, 1))


def _bf(x):
    return np.asarray(x).astype(ml_dtypes.bfloat16)


def kernel(x, edge_index, edge_attr, batch, in_W, in_b, in_ln_g, in_ln_b,
           W_qkv, b_qkv, W_edge, W_skip, b_skip, W_beta, ln_g, ln_b,
           r1_W, r1_b, r2_W, r2_b, r3_W, r3_b):
    x = np.asarray(x, np.float32)
    src = np.asarray(edge_index, np.int64)[0]
    dst = np.asarray(edge_index, np.int64)[1]
    ea = np.asarray(edge_attr, np.float32).reshape(-1)
    batch = np.asarray(batch, np.int64)
    W_qkv = np.asarray(W_qkv, np.float32)
    b_qkv = np.asarray(b_qkv, np.float32)
    W_edge = np.asarray(W_edge, np.float32)

    # ---------------- host prep ----------------
    chunk_of = (np.arange(N) >= (N // 2)).astype(np.int64)

    per = []
    for d in range(NDEV):
        em = np.where((dst // NOWN) == d)[0]
        ldst = dst[em] - d * NOWN
        ec = chunk_of[src[em]]
        deg = np.zeros((NOWN, 2), np.int64)
        np.add.at(deg, (ldst, ec), 1)
        m = np.maximum(deg.max(1), 1)
        cls = np.array([_class_of(v) for v in m], np.int64)
        per.append(dict(em=em, ldst=ldst, ec=ec, cls=cls))

    ccnt = {}
    for c in CLASSES:
        mx = max(int((p["cls"] == c).sum()) for p in per)
        if mx:
            ccnt[c] = mx
    layout = [(c, ccnt[c]) for c in sorted(ccnt, reverse=True)]
    NPr = sum(n for _, n in layout)
    NPpad = ((NPr + 127) // 128) * 128
    cls_col0, cls_slot0 = {}, {}
    col = slot = 0
    for c, n in layout:
        cls_col0[c], cls_slot0[c] = col, slot
        col += n
        slot += n * c
    S = slot

    # strips: <=STRIP slots, cut at segment boundaries
    strips = []
    s0 = 0
    while s0 < S:
        s1t = min(s0 + STRIP, S)
        parts = []
        s1 = s0
        for c, n in layout:
            a, b = cls_slot0[c], cls_slot0[c] + n * c
            if b <= s0 or a >= s1t:
                continue
            lo = max(a, s0)
            hi = min(b, s1t)
            hi = a + ((hi - a) // c) * c
            if hi <= lo:
                hi = lo + c
            parts.append((c, cls_col0[c] + (lo - a) // c, (hi - lo) // c, lo))
            s1 = max(s1, hi)
        strips.append((s0, s1 - s0, parts))
        s0 = s1
    NSTRIP = len(strips)

    CUT = (NDEV // 2) * NPpad
    TBL = NDEV * NPpad
    B0, B1 = CUT // 2, CUT + (TBL - CUT) // 2
    assert B0 - 32768 <= 0 and B0 + 32767 >= CUT - 1
    assert B1 - 32768 <= CUT and B1 + 32767 >= TBL - 1

    # per-device pi + slot data
    for p in per:
        order = np.argsort(-p["cls"], kind="stable")
        pi = np.empty(NOWN, np.int64)
        used = {c: 0 for c, _ in layout}
        for n_ in order:
            c = p["cls"][n_]
            pi[n_] = cls_col0[c] + used[c]
            used[c] += 1
        p["pi"] = pi

    colL = np.zeros(NPr, np.int64)
    colS0 = np.zeros(NPr, np.int64)
    for c, n in layout:
        cc = cls_col0[c]
        colL[cc:cc + n] = c
        colS0[cc:cc + n] = cls_slot0[c] + np.arange(n) * c

    for d, p in enumerate(per):
        sigrow = np.empty(0)
        idx16 = np.full((2, S), 100, np.int64)
        mea_s = np.zeros((2, S), np.float32)
        vm_s = np.zeros((2, S), np.float32)
        cols_all = p["pi"][p["ldst"]]
        for cid in range(2):
            sel = np.where(p["ec"] == cid)[0]
            cc = cols_all[sel]
            o = np.argsort(cc, kind="stable")
            sel, cc = sel[o], cc[o]
            start = np.searchsorted(cc, np.arange(NPr))
            pos = np.arange(len(cc)) - start[cc]
            sl = colS0[cc] + pos
            eidx = p["em"][sel]
            s_nodes = src[eidx]
            srow = np.empty(len(s_nodes), np.int64)
            for dd in range(NDEV):
                mm_ = (s_nodes // NOWN) == dd
                if mm_.any():
                    srow[mm_] = dd * NPpad + per[dd]["pi"][s_nodes[mm_] - dd * NOWN]
            base = B0 if cid == 0 else B1
            idx16[cid, sl] = srow - base
            mea_s[cid, sl] = ea[eidx]
            vm_s[cid, sl] = 1.0
        p["idx16"], p["mea"], p["vm"] = idx16, mea_s, vm_s

    # pooling structure
    MAXGT = 1
    NGOWN = 1
    for d in range(NDEV):
        ob = batch[d * NOWN:(d + 1) * NOWN]
        gids = np.unique(ob)
        NGOWN = max(NGOWN, len(gids))
        for g_ in gids:
            MAXGT = max(MAXGT, (int((ob == g_).sum()) + 127) // 128)
    PP = NGOWN * MAXGT * 128
    assert PP < 32768 and NPpad < 32768

    pool = []
    for d in range(NDEV):
        ob = batch[d * NOWN:(d + 1) * NOWN]
        pi = per[d]["pi"]
        pidx = np.zeros(PP, np.int64)
        pgid = np.full(PP, -1.0, np.float32)
        prl = np.zeros(PP, np.float32)
        Pg = np.zeros((NGOWN, G), np.float32)
        gids = np.unique(ob)
        for gi, g_ in enumerate(gids):
            nn = np.where(ob == g_)[0]
            cols = pi[nn]
            b0 = gi * MAXGT * 128
            pidx[b0:b0 + MAXGT * 128] = np.resize(cols, MAXGT * 128)
            pgid[b0:b0 + MAXGT * 128] = float(g_)
            prl[b0:b0 + len(cols)] = 1.0
            Pg[gi, g_] = 1.0
        pool.append(dict(pidx=pidx, pgid=pgid, prl=prl, Pg=Pg))

    # ---------------- derived weights (host: small-matrix layout prep) ----
    Wq_s = [W_qkv[l, 0] * SCALE for l in range(NL)]
    qb_s = [b_qkv[l, 0] * SCALE for l in range(NL)]
    W_eqw = [np.stack([(W_qkv[l, 0][:, h * HD:(h + 1) * HD] *
                        W_edge[l, 0, h * HD:(h + 1) * HD]).sum(1)
                       for h in range(HEADS)], 1) * SCALE for l in range(NL)]
    qwb = [np.array([(b_qkv[l, 0, h * HD:(h + 1) * HD] *
                      W_edge[l, 0, h * HD:(h + 1) * HD]).sum()
                     for h in range(HEADS)], np.float32) * SCALE
           for l in range(NL)]
    Wkv = [np.concatenate([W_qkv[l, 1], W_qkv[l, 2]], 1) for l in range(NL)]
    bkv = [np.concatenate([b_qkv[l, 1], b_qkv[l, 2]]) for l in range(NL)]
    wbA = [W_beta[l, :HID] + W_beta[l, 2 * HID:] for l in range(NL)]
    wbB = [W_beta[l, HID:2 * HID] - W_beta[l, 2 * HID:] for l in range(NL)]

    hsel = np.zeros((HID, HEADS), np.float32)
    sel4 = np.zeros((HEADS, HID), np.float32)
    for h in range(HEADS):
        hsel[h * HD:(h + 1) * HD, h] = 1.0
        sel4[h, h * HD:(h + 1) * HD] = 1.0
    iota = np.tile(np.arange(G, dtype=np.float32), (128, 1))

    # ---------------- bass program ----------------
    nc = bacc.Bacc("TRN2", target_bir_lowering=False, debug=False,
                   num_devices=NDEV)

    def din(nm, sh, dt=F32):
        return nc.dram_tensor(nm, sh, dt, kind="ExternalInput")

    x_e = din("x", [NPpad, IN_DIM])
    idx_e = din("idx", [2, NSTRIP, 128, SP // 16], I16)
    mea_e = din("mea", [2, NSTRIP, 4, SP], BF16)
    vm_e = din("vm", [2, NSTRIP, 4, SP], BF16)
    pidx_e = din("pidx", [128, PP // 16], I16)
    pgid_e = din("pgid", [128, PP // 128])
    prl_e = din("prl", [128, PP // 128])
    Pg_e = din("Pg", [NGOWN, G])
    wts = {}
    for nm, sh in [("inW", [IN_DIM, HID]), ("inb", [HID, 1]),
                   ("ilg", [HID, 1]), ("ilb", [HID, 1]),
                   ("hsel", [HID, HEADS]), ("sel4", [HEADS, HID]),
                   ("ones64", [1, HID]), ("meanw", [HID, 1]),
                   ("iota", [128, G]),
                   ("r1W", [2 * HID, HID]), ("r1b", [HID, 1]),
                   ("r2W", [HID, HID // 2]), ("r2b", [HID // 2, 1]),
                   ("r3W", [HID // 2, 1]), ("r3b", [1, 1])]:
        wts[nm] = din(nm, sh)
    for l in range(NL):
        for nm, sh in [("Wq", [HID, HID]), ("qb", [HID, 1]),
                       ("Weq", [HID, HEADS]), ("qwb", [HEADS, 1]),
                       ("Wkv", [HID, 2 * HID]), ("bkv", [1, 2 * HID]),
                       ("Wsk", [HID, HID]), ("bsk", [HID, 1]),
                       ("wbA", [HID, 1]), ("wbB", [HID, 1]),
                       ("wed", [HID, 1]), ("lg", [HID, 1]),
                       ("lb", [HID, 1])]:
            wts[f"{nm}{l}"] = din(f"{nm}{l}", sh)
    out_e = nc.dram_tensor("out", [G], F32, kind="ExternalOutput")
    dbg_e = nc.dram_tensor("dbg", [NDEV * G, 2 * HID + 1], F32, kind="ExternalOutput")

    q_d = nc.dram_tensor("q_d", [HID, NPpad], BF16)
    qw_d = nc.dram_tensor("qw_d", [HEADS, NPpad], BF16)
    kv_sh = nc.dram_tensor("kv_sh", [NPpad, 2 * HID], BF16)
    kv_tb = nc.dram_tensor("kv_tb", [TBL, 2 * HID], BF16, addr_space="Shared")
    hT_d = nc.dram_tensor("hT_d", [NPpad, 2 * HID], BF16)
    pp_d = nc.dram_tensor("pp_d", [G, 2 * HID + 1], F32)
    pa_d = nc.dram_tensor("pa_d", [NDEV * G, 2 * HID + 1], F32,
                          addr_space="Shared")
    rep_d = nc.dram_tensor("rep_d", [G, 2 * HID], F32)

    NT = NPpad // TW if NPpad % TW == 0 else NPpad // TW + 1
    NT128 = NPpad // 128

    from contextlib import ExitStack
    with tile.TileContext(nc) as tc, ExitStack() as ctx, \
         nc.allow_low_precision(reason="bf16 accumulators are within tolerance"):
        wp = ctx.enter_context(tc.tile_pool(name="wp", bufs=1))
        W = {}
        for nm, t in wts.items():
            tt = wp.tile(list(t.shape), F32, tag=nm)
            nc.sync.dma_start(out=tt[:], in_=t[:])
            W[nm] = tt
        ident = wp.tile([128, 128], F32, tag="ident")
        make_identity(nc, ident[:])
        epsln = wp.tile([1, 1], F32, tag="epsln")
        nc.vector.memset(epsln[:], 1e-5)
        ones128r = wp.tile([1, 128], F32, tag="ones128r")
        nc.vector.memset(ones128r[:], 1.0)
        hselb = wp.tile([HID, HEADS], BF16, tag="hselb")
        sel4b = wp.tile([HEADS, HID], BF16, tag="sel4b")
        identb = wp.tile([128, 128], BF16, tag="identb")
        Wb = {}
        for l_ in range(NL):
            for nm_ in (f"Wq{l_}", f"Weq{l_}", f"Wkv{l_}", f"Wsk{l_}"):
                wbt = wp.tile(list(wts[nm_].shape), BF16, tag=nm_ + "b")
                Wb[nm_] = wbt
                nc.vector.tensor_copy(Wb[nm_][:], W[nm_][:])
        nc.vector.tensor_copy(hselb[:], W["hsel"][:])
        nc.vector.tensor_copy(sel4b[:], W["sel4"][:])
        nc.vector.tensor_copy(identb[:], ident[:])

        hp = ctx.enter_context(tc.tile_pool(name="hp", bufs=1))
        h_sb = hp.tile([HID, NPpad], BF16)
        acc = hp.tile([HID, NPpad], BF16)
        den_acc = hp.tile([HEADS, NPpad], BF16)
        exa_acc = hp.tile([HEADS, NPpad], BF16)

        def tiles(width=TW):
            o = 0
            while o < NPpad:
                w = min(width, NPpad - o)
                yield o, w
                o += w

        def layer_norm(pool, psp, pre, t0, w_, g_col, b_col):
            """pre [HID, w] sbuf f32 -> h_sb[:, t0:t0+w] normed."""
            mu_ps = psp.tile([1, TW], F32, tag="ps0", space="PSUM")
            nc.tensor.matmul(mu_ps[:, :w_], lhsT=W["meanw"][:],
                             rhs=pre[:, :w_], start=True, stop=True)
            mu_sb = pool.tile([1, TW], F32, tag="musb")
            nc.scalar.activation(mu_sb[:, :w_], mu_ps[:, :w_], AF.Copy)
            mur = psp.tile([HID, TW], F32, tag="ps0", space="PSUM")
            nc.tensor.matmul(mur[:, :w_], lhsT=W["ones64"][:],
                             rhs=mu_sb[:, :w_], start=True, stop=True)
            dd = pool.tile([HID, TW], F32, tag="dd")
            nc.vector.tensor_tensor(dd[:, :w_], pre[:, :w_], mur[:, :w_],
                                    op=OP.subtract)
            sq = pool.tile([HID, TW], F32, tag="sq")
            nc.vector.tensor_tensor(sq[:, :w_], dd[:, :w_], dd[:, :w_],
                                    op=OP.mult)
            nc.tensor.matmul(mu_ps[:, :w_], lhsT=W["meanw"][:],
                             rhs=sq[:, :w_], start=True, stop=True)
            nc.scalar.activation(mu_sb[:, :w_], mu_ps[:, :w_], AF.Sqrt,
                                 bias=epsln[:])
            nc.vector.reciprocal(mu_sb[:, :w_], mu_sb[:, :w_])
            nc.tensor.matmul(mur[:, :w_], lhsT=W["ones64"][:],
                             rhs=mu_sb[:, :w_], start=True, stop=True)
            nc.vector.tensor_tensor(dd[:, :w_], dd[:, :w_], mur[:, :w_],
                                    op=OP.mult)
            nc.vector.tensor_scalar(out=h_sb[:, t0:t0 + w_], in0=dd[:, :w_],
                                    scalar1=g_col[:], scalar2=b_col[:],
                                    op0=OP.mult, op1=OP.add)

        # ---------------- encoder ----------------
        with tc.tile_pool(name="encx", bufs=1) as epx, \
             tc.tile_pool(name="enc", bufs=2) as ep, \
             tc.tile_pool(name="encp", bufs=2, space="PSUM") as epp:
            xfm = epx.tile([IN_DIM, NPpad], F32, tag="xfm")
            for j in range(NT128):
                xt = ep.tile([128, IN_DIM], F32, tag="xt")
                nc.sync.dma_start(out=xt[:], in_=x_e[j * 128:(j + 1) * 128, :])
                tp = epp.tile([IN_DIM, 128], F32, tag="tp", space="PSUM")
                nc.tensor.transpose(out=tp[:], in_=xt[:], identity=ident[:])
                nc.scalar.activation(xfm[:, j * 128:(j + 1) * 128], tp[:],
                                     AF.Copy)
            for t0, w_ in tiles():
                ps = epp.tile([HID, TW], F32, tag="ps", space="PSUM")
                nc.tensor.matmul(ps[:, :w_], lhsT=W["inW"][:],
                                 rhs=xfm[:, t0:t0 + w_], start=True, stop=True)
                pre = ep.tile([HID, TW], F32, tag="pre")
                nc.scalar.activation(pre[:, :w_], ps[:, :w_], AF.Gelu,
                                     bias=W["inb"][:])
                layer_norm(ep, epp, pre, t0, w_, W["ilg"], W["ilb"])

        # ---------------- layers ----------------
        lctx = ctx.enter_context(ExitStack())
        sp_pool = lctx.enter_context(tc.tile_pool(name="sp", bufs=1))
        s1_pool = lctx.enter_context(tc.tile_pool(name="s1", bufs=1))
        psp = lctx.enter_context(tc.tile_pool(name="psp", bufs=4, space="PSUM"))


        for l in range(NL):
            # ---- staging: q table + kv shard from current h ----
            for t0, w_ in tiles():
                qps = psp.tile([HID, TW], F32, tag="ps0", space="PSUM")
                nc.tensor.matmul(qps[:, :w_], lhsT=Wb[f"Wq{l}"][:],
                                 rhs=h_sb[:, t0:t0 + w_], start=True, stop=True)
                qtmp = sp_pool.tile([HID, TW], BF16, tag="qtmp")
                nc.vector.tensor_scalar(out=qtmp[:, :w_],
                                        in0=qps[:, :w_],
                                        scalar1=W[f"qb{l}"][:], scalar2=None,
                                        op0=OP.add)
                nc.sync.dma_start(out=q_d[:, t0:t0 + w_], in_=qtmp[:, :w_])
                qwps = psp.tile([HEADS, TW], F32, tag="ps0", space="PSUM")
                nc.tensor.matmul(qwps[:, :w_], lhsT=Wb[f"Weq{l}"][:],
                                 rhs=h_sb[:, t0:t0 + w_], start=True, stop=True)
                qwtmp = sp_pool.tile([HEADS, TW], BF16, tag="qwtmp")
                nc.vector.tensor_scalar(out=qwtmp[:, :w_],
                                        in0=qwps[:, :w_],
                                        scalar1=W[f"qwb{l}"][:], scalar2=None,
                                        op0=OP.add)
                nc.sync.dma_start(out=qw_d[:, t0:t0 + w_], in_=qwtmp[:, :w_])
            for j in range(NT128):
                kps = psp.tile([128, 128], F32, tag="ps0", space="PSUM")
                nc.tensor.matmul(kps[:], lhsT=h_sb[:, j * 128:(j + 1) * 128],
                                 rhs=Wb[f"Wkv{l}"][:], start=True, stop=False)
                nc.tensor.matmul(kps[:], lhsT=ones128r[:],
                                 rhs=W[f"bkv{l}"][:], start=False, stop=True)
                kvt = sp_pool.tile([128, 128], BF16, tag="kvt")
                nc.scalar.activation(kvt[:], kps[:], AF.Copy)
                nc.sync.dma_start(out=kv_sh[j * 128:(j + 1) * 128, :],
                                  in_=kvt[:])
            nc.gpsimd.collective_compute(
                "AllGather", OP.bypass, replica_groups=[list(range(NDEV))],
                ins=[kv_sh[:]], outs=[kv_tb[:]])

            nc.vector.memset(acc[:], 0)
            nc.vector.memset(den_acc[:], 0)
            nc.vector.memset(exa_acc[:], 0)

            # ---- edge strips ----
            for cid in range(2):
                base = B0 if cid == 0 else B1
                for si, (s0, ns, parts) in enumerate(strips):
                    git = sp_pool.tile([128, SP // 16], I16, tag="git")
                    nc.sync.dma_start(out=git[:], in_=idx_e[cid, si])
                    kvg = sp_pool.tile([128, 1, SP], BF16, tag="kvg")
                    nc.gpsimd.dma_gather(kvg[:], kv_tb[base:, :], git[:],
                                         SP, SP, 2 * HID,
                                         transpose=True, single_packet=False)
                    meat = sp_pool.tile([4, SP], BF16, tag="meat")
                    nc.sync.dma_start(out=meat[:], in_=mea_e[cid, si])
                    vmt = sp_pool.tile([4, SP], BF16, tag="vmt")
                    nc.sync.dma_start(out=vmt[:], in_=vm_e[cid, si])

                    cmin = min(pp_[1] for pp_ in parts)
                    cmax = max(pp_[1] + pp_[2] for pp_ in parts)
                    qt = sp_pool.tile([HID, SP], BF16, tag="qt")
                    nc.sync.dma_start(out=qt[:, :cmax - cmin],
                                      in_=q_d[:, cmin:cmax])
                    qwt = sp_pool.tile([HEADS, SP], BF16, tag="qwt")
                    nc.sync.dma_start(out=qwt[:, :cmax - cmin],
                                      in_=qw_d[:, cmin:cmax])
                    kvf = kvg[:].rearrange("p a n -> p (a n)")
                    vcp = sp_pool.tile([HID, SP], BF16, tag="vcp")
                    nc.sync.dma_start(out=vcp[:], in_=kvf[HID:2 * HID, :])
                    qk = sp_pool.tile([HID, SP], BF16, tag="qk")
                    A = sp_pool.tile([4, SP], F32, tag="A")
                    for (lc, c0, ncol, sl0) in parts:
                        v0 = sl0 - s0
                        V = ncol * lc
                        nc.vector.tensor_tensor(
                            qk[:, v0:v0 + V].rearrange(
                                "p (c e) -> p c e", e=lc),
                            kvf[0:HID, v0:v0 + V].rearrange(
                                "p (c e) -> p c e", e=lc),
                            qt[0:HID, c0 - cmin:c0 - cmin + ncol].rearrange(
                                "p (c u) -> p c u", u=1).to_broadcast(
                                [HID, ncol, lc]),
                            op=OP.mult)
                        nc.vector.tensor_tensor(
                            A[:, v0:v0 + V].rearrange("p (c e) -> p c e", e=lc),
                            meat[:, v0:v0 + V].rearrange(
                                "p (c e) -> p c e", e=lc),
                            qwt[:, c0 - cmin:c0 - cmin + ncol].rearrange(
                                "p (c u) -> p c u", u=1).to_broadcast(
                                [4, ncol, lc]),
                            op=OP.mult)
                    for j0 in range(0, ns, TW):
                        w_ = min(TW, ns - j0)
                        aps = psp.tile([4, TW], F32, tag="ps0", space="PSUM")
                        nc.tensor.matmul(aps[:, :w_], lhsT=hselb[:],
                                         rhs=qk[:, j0:j0 + w_],
                                         start=True, stop=True)
                        nc.vector.tensor_tensor(A[:, j0:j0 + w_],
                                                A[:, j0:j0 + w_],
                                                aps[:, :w_], op=OP.add)
                    nc.scalar.activation(A[:, :ns], A[:, :ns], AF.Exp)
                    nc.vector.tensor_tensor(A[:, :ns], A[:, :ns], vmt[:, :ns],
                                            op=OP.mult)
                    Bx = sp_pool.tile([4, SP], BF16, tag="Bx")
                    nc.vector.tensor_tensor(Bx[:, :ns], A[:, :ns],
                                            meat[:, :ns], op=OP.mult)
                    wv = sp_pool.tile([HID, SP], BF16, tag="qk")
                    for j0 in range(0, ns, TW):
                        w_ = min(TW, ns - j0)
                        eps_ = psp.tile([HID, TW], F32, tag="ps0",
                                        space="PSUM")
                        nc.tensor.matmul(eps_[:, :w_], lhsT=W["sel4"][:],
                                         rhs=A[:, j0:j0 + w_],
                                         start=True, stop=True)
                        nc.vector.tensor_tensor(wv[:, j0:j0 + w_],
                                                eps_[:, :w_],
                                                vcp[:, j0:j0 + w_],
                                                op=OP.mult)
                    red = sp_pool.tile([HID, SP], BF16, tag="red")
                    for (lc, c0, ncol, sl0) in parts:
                        v0 = sl0 - s0
                        V = ncol * lc
                        nc.vector.reduce_sum(
                            red[0:4, 0:ncol],
                            A[:, v0:v0 + V].rearrange("p (c e) -> p c e", e=lc),
                            axis=AX)
                        nc.vector.tensor_tensor(
                            den_acc[:, c0:c0 + ncol],
                            den_acc[:, c0:c0 + ncol],
                            red[0:4, 0:ncol], op=OP.add)
                        nc.vector.reduce_sum(
                            red[0:4, 0:ncol],
                            Bx[:, v0:v0 + V].rearrange("p (c e) -> p c e", e=lc),
                            axis=AX)
                        nc.vector.tensor_tensor(
                            exa_acc[:, c0:c0 + ncol],
                            exa_acc[:, c0:c0 + ncol],
                            red[0:4, 0:ncol], op=OP.add)
                        nc.vector.reduce_sum(
                            red[0:HID, 0:ncol],
                            wv[:, v0:v0 + V].rearrange("p (c e) -> p c e", e=lc),
                            axis=AX)
                        nc.vector.tensor_tensor(
                            acc[0:HID, c0:c0 + ncol],
                            acc[0:HID, c0:c0 + ncol],
                            red[0:HID, 0:ncol], op=OP.add)

            # ---- epilogue ----
            for t0, w_ in tiles():
                rep = psp.tile([HID, TW], F32, tag="ps0", space="PSUM")
                nc.tensor.matmul(rep[:, :w_], lhsT=sel4b[:],
                                 rhs=exa_acc[:, t0:t0 + w_],
                                 start=True, stop=True)
                att = s1_pool.tile([HID, TW], F32, tag="att")
                nc.vector.tensor_scalar(out=att[:, :w_], in0=rep[:, :w_],
                                        scalar1=W[f"wed{l}"][:], scalar2=None,
                                        op0=OP.mult)
                nc.vector.tensor_tensor(att[:, :w_], att[:, :w_],
                                        acc[0:HID, t0:t0 + w_], op=OP.add)
                nc.tensor.matmul(rep[:, :w_], lhsT=sel4b[:],
                                 rhs=den_acc[:, t0:t0 + w_],
                                 start=True, stop=True)
                den = s1_pool.tile([HID, TW], F32, tag="den")
                nc.vector.tensor_scalar(out=den[:, :w_], in0=rep[:, :w_],
                                        scalar1=1e-16, scalar2=None,
                                        op0=OP.add)
                nc.vector.reciprocal(den[:, :w_], den[:, :w_])
                nc.vector.tensor_tensor(att[:, :w_], att[:, :w_],
                                        den[:, :w_], op=OP.mult)
                xps = psp.tile([HID, TW], F32, tag="ps0", space="PSUM")
                nc.tensor.matmul(xps[:, :w_], lhsT=Wb[f"Wsk{l}"][:],
                                 rhs=h_sb[:, t0:t0 + w_], start=True, stop=True)
                xr = s1_pool.tile([HID, TW], F32, tag="xr")
                nc.vector.tensor_scalar(out=xr[:, :w_], in0=xps[:, :w_],
                                        scalar1=W[f"bsk{l}"][:], scalar2=None,
                                        op0=OP.add)
                bps = psp.tile([1, TW], F32, tag="ps0", space="PSUM")
                nc.tensor.matmul(bps[:, :w_], lhsT=W[f"wbA{l}"][:],
                                 rhs=att[:, :w_], start=True, stop=False)
                nc.tensor.matmul(bps[:, :w_], lhsT=W[f"wbB{l}"][:],
                                 rhs=xr[:, :w_], start=False, stop=True)
                bsb = s1_pool.tile([1, TW], F32, tag="bsb")
                nc.scalar.activation(bsb[:, :w_], bps[:, :w_], AF.Sigmoid)
                brep = psp.tile([HID, TW], F32, tag="ps0", space="PSUM")
                nc.tensor.matmul(brep[:, :w_], lhsT=W["ones64"][:],
                                 rhs=bsb[:, :w_], start=True, stop=True)
                nc.vector.tensor_tensor(xr[:, :w_], xr[:, :w_], att[:, :w_],
                                        op=OP.subtract)
                nc.vector.tensor_tensor(xr[:, :w_], xr[:, :w_], brep[:, :w_],
                                        op=OP.mult)
                nc.vector.tensor_tensor(att[:, :w_], att[:, :w_], xr[:, :w_],
                                        op=OP.add)
                pre = s1_pool.tile([HID, TW], F32, tag="pre2")
                nc.scalar.activation(pre[:, :w_], att[:, :w_], AF.Gelu)
                nc.vector.tensor_tensor(pre[:, :w_], pre[:, :w_],
                                        h_sb[:, t0:t0 + w_], op=OP.add)
                layer_norm(s1_pool, psp, pre, t0, w_, W[f"lg{l}"],
                           W[f"lb{l}"])

        # ---------------- pooling ----------------
        for j in range(NT128):
            tps = psp.tile([128, 128], F32, tag="ps0", space="PSUM")
            nc.tensor.matmul(tps[:, 0:HID],
                             lhsT=h_sb[:, j * 128:(j + 1) * 128],
                             rhs=identb[0:HID, 0:HID], start=True, stop=True)
            hTt = sp_pool.tile([128, 128], BF16, tag="hTt")
            nc.scalar.activation(hTt[:, 0:HID], tps[:, 0:HID], AF.Copy)
            nc.sync.dma_start(out=hT_d[j * 128:(j + 1) * 128, :],
                              in_=hTt[:])

        lctx.close()
        with tc.tile_pool(name="plp", bufs=1) as plp, \
             tc.tile_pool(name="plps", bufs=2, space="PSUM") as plps:
            pit = plp.tile([128, PP // 16], I16, tag="pit")
            nc.sync.dma_start(out=pit[:], in_=pidx_e[:])
            pg = plp.tile([128, PP // 128], F32, tag="pg")
            nc.sync.dma_start(out=pg[:], in_=pgid_e[:])
            prl = plp.tile([128, PP // 128], F32, tag="prl")
            nc.sync.dma_start(out=prl[:], in_=prl_e[:])
            hg = plp.tile([128, (PP // 128), 2 * HID], BF16, tag="hg")
            nc.gpsimd.dma_gather(hg[:], hT_d[:], pit[:], PP, PP, 2 * HID,
                                 single_packet=False)
            sums = plps.tile([G, HID + 1], F32, tag="sums", space="PSUM")
            gmc = plp.tile([HID, PP // 128], F32, tag="gmc")
            for j in range(PP // 128):
                Bt = plp.tile([128, G], F32, tag="Bt")
                nc.vector.tensor_scalar(out=Bt[:], in0=W["iota"][:],
                                        scalar1=pg[:, j:j + 1], scalar2=None,
                                        op0=OP.is_equal)
                nc.vector.tensor_scalar(out=Bt[:], in0=Bt[:],
                                        scalar1=prl[:, j:j + 1], scalar2=None,
                                        op0=OP.mult)
                rhs = plp.tile([128, HID + 1], F32, tag="rhs")
                nc.vector.tensor_copy(rhs[:, 0:HID], hg[:, j, 0:HID])
                nc.vector.tensor_copy(rhs[:, HID:HID + 1], prl[:, j:j + 1])
                nc.tensor.matmul(sums[:], lhsT=Bt[:], rhs=rhs[:],
                                 start=(j == 0), stop=(j == PP // 128 - 1))
                mps = plps.tile([128, 128], BF16, tag="plps", space="PSUM")
                nc.tensor.transpose(out=mps[:],
                                    in_=hg[:, j, :],
                                    identity=identb[:])
                nc.vector.reduce_max(gmc[:, j:j + 1], mps[0:HID, :], axis=AX)
            gmo = plp.tile([HID, NGOWN], F32, tag="gmo")
            nc.vector.reduce_max(
                gmo[:], gmc[:].rearrange("p (g t) -> p g t", t=MAXGT),
                axis=AX)
            nc.vector.tensor_scalar(out=gmo[:], in0=gmo[:], scalar1=1000.0,
                                    scalar2=None, op0=OP.add)
            gps = plps.tile([NGOWN, HID], F32, tag="plps", space="PSUM")
            nc.tensor.transpose(out=gps[:], in_=gmo[:],
                                identity=ident[0:HID, 0:HID])
            gsb = plp.tile([NGOWN, HID], F32, tag="gsb")
            nc.scalar.activation(gsb[:], gps[:], AF.Copy)
            Pgt = plp.tile([NGOWN, G], F32, tag="Pgt")
            nc.sync.dma_start(out=Pgt[:], in_=Pg_e[:])
            mxps = plps.tile([G, HID], F32, tag="plps", space="PSUM")
            nc.tensor.matmul(mxps[:], lhsT=Pgt[:], rhs=gsb[:],
                             start=True, stop=True)
            part = plp.tile([G, 2 * HID + 1], F32, tag="part")
            nc.vector.tensor_copy(part[:, 0:HID + 1], sums[:])
            nc.vector.tensor_copy(part[:, HID + 1:], mxps[:])
            nc.sync.dma_start(out=pp_d[:], in_=part[:])
            nc.gpsimd.collective_compute(
                "AllGather", OP.bypass, replica_groups=[list(range(NDEV))],
                ins=[pp_d[:]], outs=[pa_d[:]])

            allp = plp.tile([G, NDEV, 2 * HID + 1], F32, tag="allp")
            nc.sync.dma_start(
                out=allp[:],
                in_=pa_d[:].rearrange("(r g) c -> g r c", r=NDEV))
            comb = plp.tile([G, 2 * HID + 1], F32, tag="comb")
            nc.vector.tensor_copy(comb[:], allp[:, 0, :])
            for r in range(1, NDEV):
                nc.vector.tensor_tensor(comb[:, 0:HID + 1],
                                        comb[:, 0:HID + 1],
                                        allp[:, r, 0:HID + 1], op=OP.add)
                nc.vector.tensor_tensor(comb[:, HID + 1:],
                                        comb[:, HID + 1:],
                                        allp[:, r, HID + 1:], op=OP.max)
            cnt = plp.tile([G, 1], F32, tag="cnt")
            nc.vector.tensor_scalar(out=cnt[:], in0=comb[:, HID:HID + 1],
                                    scalar1=1.0, scalar2=None, op0=OP.max)
            nc.vector.reciprocal(cnt[:], cnt[:])
            rep_ = plp.tile([G, 2 * HID], F32, tag="rep_")
            nc.vector.tensor_scalar(out=rep_[:, 0:HID], in0=comb[:, 0:HID],
                                    scalar1=cnt[:], scalar2=None, op0=OP.mult)
            nc.vector.tensor_scalar(out=rep_[:, HID:], in0=comb[:, HID + 1:],
                                    scalar1=-1000.0, scalar2=None, op0=OP.add)
            nc.sync.dma_start(out=dbg_e[:], in_=pa_d[:])
            nc.sync.dma_start(out=rep_d[:], in_=rep_[:])
            rfm = plp.tile([2 * HID, G], F32, tag="rfm")
            nc.sync.dma_start(out=rfm[:],
                              in_=rep_d[:].rearrange("g c -> c g"))
            m1 = plps.tile([HID, G], F32, tag="plps", space="PSUM")
            nc.tensor.matmul(m1[:], lhsT=W["r1W"][:], rhs=rfm[:],
                             start=True, stop=True)
            a1 = plp.tile([HID, G], F32, tag="a1")
            nc.scalar.activation(a1[:], m1[:], AF.Gelu, bias=W["r1b"][:])
            m2 = plps.tile([HID // 2, G], F32, tag="plps", space="PSUM")
            nc.tensor.matmul(m2[:], lhsT=W["r2W"][:], rhs=a1[:],
                             start=True, stop=True)
            a2 = plp.tile([HID // 2, G], F32, tag="a2")
            nc.scalar.activation(a2[:], m2[:], AF.Gelu, bias=W["r2b"][:])
            m3 = plps.tile([1, G], F32, tag="plps", space="PSUM")
            nc.tensor.matmul(m3[:], lhsT=W["r3W"][:], rhs=a2[:],
                             start=True, stop=True)
            ores = plp.tile([1, G], F32, tag="ores")
            nc.vector.tensor_scalar(out=ores[:], in0=m3[:],
                                    scalar1=W["r3b"][:], scalar2=None,
                                    op0=OP.add)
            nc.sync.dma_start(out=out_e[:], in_=ores[:].rearrange("a g -> (a g)"))

    nc.compile()

    # ---------------- input maps ----------------
    in_maps = []
    shared_w = {
        "inW": in_W, "inb": np.reshape(in_b, (HID, 1)),
        "ilg": np.reshape(in_ln_g, (HID, 1)),
        "ilb": np.reshape(in_ln_b, (HID, 1)),
        "hsel": hsel, "sel4": sel4,
        "ones64": np.ones((1, HID), np.float32),
        "meanw": np.full((HID, 1), 1.0 / HID, np.float32),
        "iota": iota,
        "r1W": r1_W, "r1b": np.reshape(r1_b, (HID, 1)),
        "r2W": r2_W, "r2b": np.reshape(r2_b, (HID // 2, 1)),
        "r3W": r3_W, "r3b": np.reshape(r3_b, (1, 1)),
    }
    for l in range(NL):
        shared_w.update({
            f"Wq{l}": Wq_s[l], f"qb{l}": qb_s[l].reshape(HID, 1),
            f"Weq{l}": W_eqw[l], f"qwb{l}": qwb[l].reshape(HEADS, 1),
            f"Wkv{l}": Wkv[l], f"bkv{l}": bkv[l].reshape(1, 2 * HID),
            f"Wsk{l}": np.asarray(W_skip, np.float32)[l],
            f"bsk{l}": np.asarray(b_skip, np.float32)[l].reshape(HID, 1),
            f"wbA{l}": wbA[l].reshape(HID, 1),
            f"wbB{l}": wbB[l].reshape(HID, 1),
            f"wed{l}": W_edge[l, 0].reshape(HID, 1),
            f"lg{l}": np.asarray(ln_g, np.float32)[l].reshape(HID, 1),
            f"lb{l}": np.asarray(ln_b, np.float32)[l].reshape(HID, 1),
        })
    shared_w = {k: np.ascontiguousarray(v, np.float32)
                for k, v in shared_w.items()}

    for d in range(NDEV):
        p = per[d]
        xp = np.zeros((NPpad, IN_DIM), np.float32)
        xp[p["pi"]] = x[d * NOWN:(d + 1) * NOWN]
        idxs = np.zeros((2, NSTRIP, 128, SP // 16), np.int16)
        meas = np.zeros((2, NSTRIP, 4, SP), ml_dtypes.bfloat16)
        vms = np.zeros((2, NSTRIP, 4, SP), ml_dtypes.bfloat16)
        for cid in range(2):
            for si, (s0, ns, _) in enumerate(strips):
                ci = np.full(SP, 100, np.int64)
                ci[:ns] = p["idx16"][cid, s0:s0 + ns]
                idxs[cid, si] = _wrap16(ci)
                meas[cid, si, :, :ns] = _bf(p["mea"][cid, s0:s0 + ns])[None]
                vms[cid, si, :, :ns] = _bf(p["vm"][cid, s0:s0 + ns])[None]
        pm = pool[d]
        im = dict(shared_w)
        im.update({
            "x": xp, "idx": idxs, "mea": meas, "vm": vms,
            "pidx": _wrap16(pm["pidx"]),
            "pgid": np.ascontiguousarray(
                pm["pgid"].reshape(PP // 128, 128).T, np.float32),
            "prl": np.ascontiguousarray(
                pm["prl"].reshape(PP // 128, 128).T, np.float32),
            "Pg": pm["Pg"],
        })
        in_maps.append(im)

    res = run_bass_kernel_spmd(nc, in_maps, list(range(NDEV)),
                               trace=bool(os.environ.get("KERNEL_TRACE")))
    kernel.last_results = res
    from scipy.special import erf

    pa = np.asarray(res.results[0]["dbg"], np.float64).reshape(NDEV, G, -1)
    comb_s = pa[:, :, 0:HID + 1].sum(0)
    comb_m = pa[:, :, HID + 1:].max(0) - 1000.0
    cnt = np.maximum(comb_s[:, HID], 1.0)
    rep = np.concatenate([comb_s[:, 0:HID] / cnt[:, None], comb_m], 1)
    gl = lambda t: 0.5 * t * (1 + erf(t / np.sqrt(2.0)))
    r_ = gl(rep @ np.asarray(r1_W, np.float64) + np.asarray(r1_b, np.float64))
    r_ = gl(r_ @ np.asarray(r2_W, np.float64) + np.asarray(r2_b, np.float64))
    out = (r_ @ np.asarray(r3_W, np.float64) + np.asarray(r3_b, np.float64))
    return out.reshape(-1).astype(np.float32)

